# revision 6
# baseline (speedup 1.0000x reference)
"""BinaryLinear Trainium2 kernel.

Computes out = x @ sign(weight).T + bias for x [8192, 4096] f32,
weight [4096, 4096] f32, bias [4096] f32.

Sharding: 2D grid over 8 NeuronCores — 4 token groups x 2 out-feature
halves (core c -> tokens [2048*(c%4)], out rows [2048*(c//4)]). This
minimizes per-core HBM traffic (x 33.6MB + w 33.6MB + out 16.8MB) vs
pure token sharding (w replicated: 100MB).

Per-core pipeline:
  1. W path: 16 slabs of 128 weight rows are cast f32->bf16 during the
     SWDGE load, XBAR-transposed into a fully SBUF-resident WT
     [128k, 32kk, 2048o] (bf16, 128KB/partition), and signed in place
     on ScalarE (scale=1e30 pushes tiny values off the LUT's zero
     neighborhood; sign(0)=0 preserved). Slabs are emitted in n-block
     order so the first output block's matmuls can start early.
  2. X path: token-pair tiles XT_g [128k, 32kk, 256t] built from two
     cast-loads + two XBAR transposes, streamed through a 2-deep pool.
     First pairs are interleaved into the W prologue.
  3. TensorE: for each (m, n-block of 512 outf): one PSUM bank
     accumulates 32 back-to-back matmuls over kk (kk-innermost, single
     accumulation chain per bank, rotating banks at chain granularity
     only) — the loop shape that measured ~101 ns/MM on HW.
  4. DVE adds the partition-broadcast bias while copying PSUM->SBUF;
     the scalar-ring HWDGE stores f32 output tiles (keeps the gpsimd
     SWDGE ring load-only).
"""

import numpy as np

import concourse.mybir as mybir
import concourse.tile as tile
from concourse import bacc
from concourse.bass import ts

P = 128
TOKENS, IN_F, OUT_F = 8192, 4096, 4096
N_CORES = 8
T_GROUPS = 4   # token groups
O_GROUPS = 2   # out-feature groups
N_TILE = 512   # output-feature block (one PSUM bank of f32)

F32 = mybir.dt.float32
BF16 = mybir.dt.bfloat16


def build_nc(t_shard=TOKENS // T_GROUPS, in_f=IN_F, out_f=OUT_F // O_GROUPS,
             repeat=1):
    m_tiles = t_shard // P       # token tiles of 128
    g_tiles = m_tiles // 2       # token-pair groups of 256
    n_tiles = out_f // N_TILE    # output blocks of 512
    ko_tiles = in_f // P         # contraction tiles of 128
    j_tiles = out_f // P         # 128-row weight slabs

    nc = bacc.Bacc(None, target_bir_lowering=False, debug=False)

    x = nc.dram_tensor("x", [t_shard, in_f], F32, kind="ExternalInput")
    w = nc.dram_tensor("weight", [out_f, in_f], F32, kind="ExternalInput")
    b = nc.dram_tensor("bias", [out_f], F32, kind="ExternalInput")
    out = nc.dram_tensor("out", [t_shard, out_f], F32, kind="ExternalOutput")

    with tile.TileContext(nc) as tc:
        with (
            tc.tile_pool(name="xt", bufs=2) as xt_pool,
            tc.tile_pool(name="wstage", bufs=4) as wstage_pool,
            tc.tile_pool(name="xstage", bufs=4) as xstage_pool,
            tc.tile_pool(name="wt", bufs=1) as wt_pool,
            tc.tile_pool(name="bias", bufs=1) as bias_pool,
            tc.tile_pool(name="out_sb", bufs=3) as out_pool,
            tc.tile_pool(name="ps", bufs=8, space="PSUM") as psum_pool,
        ):
          for _rep in range(repeat):
            wt = wt_pool.tile([P, ko_tiles, out_f], BF16, name="wt", tag="wt")

            half_k = in_f // 2
            hk_tiles = ko_tiles // 2

            def build_w_slab(j):
                # two half-slabs per 128-row slab: finer DMA granularity
                # keeps the load->transpose pipeline full
                for h in range(2):
                    slab = wstage_pool.tile(
                        [P, half_k], BF16, name="wslab", tag="ws"
                    )
                    nc.gpsimd.dma_start(slab, w[ts(j, P), ts(h, half_k)])
                    # NOTE: transposes must stay on nc.sync — issuing them
                    # on nc.scalar's HWDGE ring corrupts results on HW.
                    nc.sync.dma_start(
                        wt[:, ts(h, hk_tiles), ts(j, P)], slab,
                        transpose=True,
                    )
                    nc.scalar.activation(
                        wt[:, ts(h, hk_tiles), ts(j, P)],
                        wt[:, ts(h, hk_tiles), ts(j, P)],
                        mybir.ActivationFunctionType.Sign, scale=1.0e30,
                    )

            def build_xt(g):
                xt_g = xt_pool.tile(
                    [P, ko_tiles, 2 * P], BF16, name=f"xt{g}", tag="xt"
                )
                for i in range(2):
                    for h in range(2):
                        slab = xstage_pool.tile(
                            [P, half_k], BF16, name="xslab", tag="xs"
                        )
                        nc.gpsimd.dma_start(
                            slab, x[ts(2 * g + i, P), ts(h, half_k)]
                        )
                        nc.sync.dma_start(
                            xt_g[:, ts(h, hk_tiles), ts(i, P)], slab,
                            transpose=True,
                        )
                return xt_g

            # interleave first x-pair builds into the W prologue
            xts = {}
            for j in range(j_tiles):
                build_w_slab(j)
                if j == 3:
                    xts[0] = build_xt(0)
                elif j == 7:
                    xts[1] = build_xt(1)

            bias_rep = bias_pool.tile(
                [P, n_tiles, N_TILE], F32, name="bias_rep", tag="bias"
            )
            for n in range(n_tiles):
                nc.gpsimd.dma_start(
                    bias_rep[:, n, :],
                    b[None, ts(n, N_TILE)].broadcast_to([P, N_TILE]),
                )

            # ---- main loop: pair-group outer, (i, n) middle, kk innermost
            for g in range(g_tiles):
                if g + 2 < g_tiles:
                    xts[g + 2] = build_xt(g + 2)
                xt_g = xts.pop(g)
                for i in range(2):
                    m = 2 * g + i
                    for n in range(n_tiles):
                        ps = psum_pool.tile(
                            [P, N_TILE], F32, name="ps", tag="ps"
                        )
                        for kk in range(ko_tiles):
                            nc.tensor.matmul(
                                ps,
                                xt_g[:, kk, ts(i, P)],
                                wt[:, kk, ts(n, N_TILE)],
                                start=(kk == 0),
                                stop=(kk == ko_tiles - 1),
                            )
                        out_sb = out_pool.tile(
                            [P, N_TILE], F32, name="out_sb", tag="out_sb"
                        )
                        nc.vector.tensor_tensor(
                            out_sb, ps, bias_rep[:, n, :], mybir.AluOpType.add
                        )
                        nc.scalar.dma_start(
                            out[ts(m, P), ts(n, N_TILE)], out_sb
                        )

    nc.compile()
    return nc


_NC_CACHE = {}


def _get_nc(shape_key):
    if shape_key not in _NC_CACHE:
        _NC_CACHE[shape_key] = build_nc(*shape_key)
    return _NC_CACHE[shape_key]


def kernel(x, weight, bias, _trace=False):
    from concourse.bass_utils import run_bass_kernel_spmd

    x = np.ascontiguousarray(np.asarray(x, dtype=np.float32))
    weight = np.ascontiguousarray(np.asarray(weight, dtype=np.float32))
    bias = np.ascontiguousarray(np.asarray(bias, dtype=np.float32))

    tokens = x.shape[0]
    out_f = weight.shape[0]
    t_shard = tokens // T_GROUPS
    o_shard = out_f // O_GROUPS
    nc = _get_nc((t_shard, x.shape[1], o_shard))

    in_maps = []
    for c in range(N_CORES):
        tg, og = c % T_GROUPS, c // T_GROUPS
        in_maps.append(
            {
                "x": x[tg * t_shard : (tg + 1) * t_shard],
                "weight": weight[og * o_shard : (og + 1) * o_shard],
                "bias": bias[og * o_shard : (og + 1) * o_shard],
            }
        )
    res = run_bass_kernel_spmd(
        nc, in_maps, core_ids=list(range(N_CORES)), trace=_trace
    )
    full = np.empty((tokens, out_f), np.float32)
    for c in range(N_CORES):
        tg, og = c % T_GROUPS, c // T_GROUPS
        full[
            tg * t_shard : (tg + 1) * t_shard,
            og * o_shard : (og + 1) * o_shard,
        ] = res.results[c]["out"]
    if _trace:
        return full, res
    return full


# revision 7
# speedup vs baseline: 1.2937x; 1.2937x over previous
"""BinaryLinear Trainium2 kernel.

Computes out = x @ sign(weight).T + bias for x [8192, 4096] f32,
weight [4096, 4096] f32, bias [4096] f32.

Sharding: 2D grid over 8 NeuronCores — 4 token groups x 2 out-feature
halves (core c -> tokens [2048*(c%4)], out rows [2048*(c//4)]). This
minimizes per-core HBM traffic (x 33.6MB + w 33.6MB + out 16.8MB) vs
pure token sharding (w replicated: 100MB).

Per-core pipeline:
  1. W path: 16 slabs of 128 weight rows are cast f32->bf16 during the
     SWDGE load, XBAR-transposed into a fully SBUF-resident WT
     [128k, 32kk, 2048o] (bf16, 128KB/partition), and signed in place
     on ScalarE (scale=1e30 pushes tiny values off the LUT's zero
     neighborhood; sign(0)=0 preserved). Slabs are emitted in n-block
     order so the first output block's matmuls can start early.
  2. X path: token-pair tiles XT_g [128k, 32kk, 256t] built from two
     cast-loads + two XBAR transposes, streamed through a 2-deep pool.
     First pairs are interleaved into the W prologue.
  3. TensorE: for each (m, n-block of 512 outf): one PSUM bank
     accumulates 32 back-to-back matmuls over kk (kk-innermost, single
     accumulation chain per bank, rotating banks at chain granularity
     only) — the loop shape that measured ~101 ns/MM on HW.
  4. DVE adds the partition-broadcast bias while copying PSUM->SBUF;
     the scalar-ring HWDGE stores f32 output tiles (keeps the gpsimd
     SWDGE ring load-only).
"""

import numpy as np

import concourse.mybir as mybir
import concourse.tile as tile
from concourse import bacc
from concourse.bass import ts

P = 128
TOKENS, IN_F, OUT_F = 8192, 4096, 4096
N_CORES = 8
T_GROUPS = 4   # token groups
O_GROUPS = 2   # out-feature groups
N_TILE = 512   # output-feature block (one PSUM bank of f32)

F32 = mybir.dt.float32
BF16 = mybir.dt.bfloat16


def build_nc(t_shard=TOKENS // T_GROUPS, in_f=IN_F, out_f=OUT_F // O_GROUPS,
             repeat=1):
    m_tiles = t_shard // P       # token tiles of 128
    g_tiles = m_tiles // 2       # token-pair groups of 256
    n_tiles = out_f // N_TILE    # output blocks of 512
    ko_tiles = in_f // P         # contraction tiles of 128
    j_tiles = out_f // P         # 128-row weight slabs

    nc = bacc.Bacc(None, target_bir_lowering=False, debug=False)

    x = nc.dram_tensor("x", [t_shard, in_f], F32, kind="ExternalInput")
    w = nc.dram_tensor("weight", [out_f, in_f], F32, kind="ExternalInput")
    b = nc.dram_tensor("bias", [out_f], F32, kind="ExternalInput")
    out = nc.dram_tensor("out", [t_shard, out_f], F32, kind="ExternalOutput")

    with tile.TileContext(nc) as tc:
        with (
            tc.tile_pool(name="xt", bufs=2) as xt_pool,
            tc.tile_pool(name="wstage", bufs=2) as wstage_pool,
            tc.tile_pool(name="xstage", bufs=2) as xstage_pool,
            tc.tile_pool(name="wt", bufs=1) as wt_pool,
            tc.tile_pool(name="bias", bufs=1) as bias_pool,
            tc.tile_pool(name="out_sb", bufs=3) as out_pool,
            tc.tile_pool(name="ps", bufs=8, space="PSUM") as psum_pool,
        ):
          for _rep in range(repeat):
            wt = wt_pool.tile([P, ko_tiles, out_f], BF16, name="wt", tag="wt")

            def build_w_slab(j):
                slab = wstage_pool.tile(
                    [P, in_f], BF16, name="wslab", tag="ws"
                )
                nc.gpsimd.dma_start(slab, w[ts(j, P), :])
                # NOTE: transposes must stay on nc.sync — issuing them on
                # nc.scalar's HWDGE ring corrupts results on HW.
                nc.sync.dma_start(wt[:, :, ts(j, P)], slab, transpose=True)
                nc.scalar.activation(
                    wt[:, :, ts(j, P)], wt[:, :, ts(j, P)],
                    mybir.ActivationFunctionType.Sign, scale=1.0e30,
                )

            def build_xt(g):
                xt_g = xt_pool.tile(
                    [P, ko_tiles, 2 * P], BF16, name=f"xt{g}", tag="xt"
                )
                for i in range(2):
                    slab = xstage_pool.tile(
                        [P, in_f], BF16, name="xslab", tag="xs"
                    )
                    nc.gpsimd.dma_start(slab, x[ts(2 * g + i, P), :])
                    nc.sync.dma_start(
                        xt_g[:, :, ts(i, P)], slab, transpose=True
                    )
                return xt_g

            # interleave first x-pair builds into the W prologue
            xts = {}
            for j in range(j_tiles):
                build_w_slab(j)
                if j == 3:
                    xts[0] = build_xt(0)
                elif j == 7:
                    xts[1] = build_xt(1)

            bias_rep = bias_pool.tile(
                [P, n_tiles, N_TILE], F32, name="bias_rep", tag="bias"
            )
            for n in range(n_tiles):
                nc.gpsimd.dma_start(
                    bias_rep[:, n, :],
                    b[None, ts(n, N_TILE)].broadcast_to([P, N_TILE]),
                )

            # ---- main loop: pair-group outer, (i, n) middle, kk innermost
            for g in range(g_tiles):
                if g + 2 < g_tiles:
                    xts[g + 2] = build_xt(g + 2)
                xt_g = xts.pop(g)
                for i in range(2):
                    m = 2 * g + i
                    for n in range(n_tiles):
                        ps = psum_pool.tile(
                            [P, N_TILE], F32, name="ps", tag="ps"
                        )
                        for kk in range(ko_tiles):
                            nc.tensor.matmul(
                                ps,
                                xt_g[:, kk, ts(i, P)],
                                wt[:, kk, ts(n, N_TILE)],
                                start=(kk == 0),
                                stop=(kk == ko_tiles - 1),
                            )
                        out_sb = out_pool.tile(
                            [P, N_TILE], F32, name="out_sb", tag="out_sb"
                        )
                        nc.vector.tensor_tensor(
                            out_sb, ps, bias_rep[:, n, :], mybir.AluOpType.add
                        )
                        nc.scalar.dma_start(
                            out[ts(m, P), ts(n, N_TILE)], out_sb
                        )

    nc.compile()
    return nc


_NC_CACHE = {}


def _get_nc(shape_key):
    if shape_key not in _NC_CACHE:
        _NC_CACHE[shape_key] = build_nc(*shape_key)
    return _NC_CACHE[shape_key]


def kernel(x, weight, bias, _trace=False):
    from concourse.bass_utils import run_bass_kernel_spmd

    x = np.ascontiguousarray(np.asarray(x, dtype=np.float32))
    weight = np.ascontiguousarray(np.asarray(weight, dtype=np.float32))
    bias = np.ascontiguousarray(np.asarray(bias, dtype=np.float32))

    tokens = x.shape[0]
    out_f = weight.shape[0]
    t_shard = tokens // T_GROUPS
    o_shard = out_f // O_GROUPS
    nc = _get_nc((t_shard, x.shape[1], o_shard))

    in_maps = []
    for c in range(N_CORES):
        tg, og = c % T_GROUPS, c // T_GROUPS
        in_maps.append(
            {
                "x": x[tg * t_shard : (tg + 1) * t_shard],
                "weight": weight[og * o_shard : (og + 1) * o_shard],
                "bias": bias[og * o_shard : (og + 1) * o_shard],
            }
        )
    res = run_bass_kernel_spmd(
        nc, in_maps, core_ids=list(range(N_CORES)), trace=_trace
    )
    full = np.empty((tokens, out_f), np.float32)
    for c in range(N_CORES):
        tg, og = c % T_GROUPS, c // T_GROUPS
        full[
            tg * t_shard : (tg + 1) * t_shard,
            og * o_shard : (og + 1) * o_shard,
        ] = res.results[c]["out"]
    if _trace:
        return full, res
    return full


# revision 10
# speedup vs baseline: 1.3396x; 1.0355x over previous
"""BinaryLinear Trainium2 kernel.

Computes out = x @ sign(weight).T + bias for x [8192, 4096] f32,
weight [4096, 4096] f32, bias [4096] f32.

Sharding: data-parallel over tokens across 8 NeuronCores (1024 tokens
per core, weight/bias replicated, no collectives).

Per-core pipeline (chosen so the weight stream fully overlaps compute):
  1. X path (prologue): 8 token slabs are cast f32->bf16 during the
     SWDGE load and XBAR-transposed into a fully SBUF-resident
     XT [128k, 32kk, 1024t] (bf16, 64KB/partition).
  2. W path (streamed): per 512-outfeature block, 4 slabs of 128
     weight rows are cast-loaded, XBAR-transposed into a double-
     buffered WT_n [128k, 32kk, 512o], and signed in place on ScalarE
     (scale=1e30 pushes tiny values off the LUT's zero neighborhood;
     sign(0)=0 preserved). Matmuls for block n start as soon as WT_n
     is signed, while block n+1 loads — the weight stream never gates
     more than one block of compute.
  3. TensorE: for each (n, m): one PSUM bank accumulates 32
     back-to-back matmuls over kk (kk-innermost, one accumulation
     chain per bank, banks rotate only at chain granularity — the
     fastest loop shape measured on HW for N=512 bf16).
  4. DVE adds the partition-broadcast bias while copying PSUM->SBUF;
     the scalar-ring HWDGE stores f32 output tiles (keeps the gpsimd
     SWDGE ring load-only).

NOTE: transposes must stay on nc.sync (HWDGE) — issuing them on
nc.scalar's ring corrupts results on HW. Mixed-dtype matmul operands
(bf16 x fp8) hard-wedge the PE (NRT_EXEC_UNIT_UNRECOVERABLE) — both
operands must share a dtype.
"""

import numpy as np

import concourse.mybir as mybir
import concourse.tile as tile
from concourse import bacc
from concourse.bass import ts

P = 128
TOKENS, IN_F, OUT_F = 8192, 4096, 4096
N_CORES = 8
T_GROUPS = 8   # token groups
O_GROUPS = 1   # out-feature groups (weight replicated)
N_TILE = 512   # output-feature block (one PSUM bank of f32)

F32 = mybir.dt.float32
BF16 = mybir.dt.bfloat16


def build_nc(t_shard=TOKENS // T_GROUPS, in_f=IN_F, out_f=OUT_F // O_GROUPS,
             repeat=1):
    m_tiles = t_shard // P       # token tiles of 128
    n_tiles = out_f // N_TILE    # output blocks of 512
    ko_tiles = in_f // P         # contraction tiles of 128
    j_tiles = N_TILE // P        # 128-row weight slabs per block

    nc = bacc.Bacc(None, target_bir_lowering=False, debug=False)

    x = nc.dram_tensor("x", [t_shard, in_f], F32, kind="ExternalInput")
    w = nc.dram_tensor("weight", [out_f, in_f], F32, kind="ExternalInput")
    b = nc.dram_tensor("bias", [out_f], F32, kind="ExternalInput")
    out = nc.dram_tensor("out", [t_shard, out_f], F32, kind="ExternalOutput")

    with tile.TileContext(nc) as tc:
        with (
            tc.tile_pool(name="xt", bufs=1) as xt_pool,
            tc.tile_pool(name="wstage", bufs=3) as wstage_pool,
            tc.tile_pool(name="xstage", bufs=3) as xstage_pool,
            tc.tile_pool(name="wtn", bufs=2) as wtn_pool,
            tc.tile_pool(name="bias", bufs=1) as bias_pool,
            tc.tile_pool(name="out_sb", bufs=3) as out_pool,
            tc.tile_pool(name="ps", bufs=8, space="PSUM") as psum_pool,
        ):
          for _rep in range(repeat):

            def build_wt_block(n):
                wt_n = wtn_pool.tile(
                    [P, ko_tiles, N_TILE], BF16, name=f"wt{n}", tag="wtn"
                )
                for j in range(j_tiles):
                    slab = wstage_pool.tile(
                        [P, in_f], BF16, name="wslab", tag="ws"
                    )
                    nc.gpsimd.dma_start(slab, w[ts(n * j_tiles + j, P), :])
                    nc.sync.dma_start(
                        wt_n[:, :, ts(j, P)], slab, transpose=True
                    )
                    nc.scalar.activation(
                        wt_n[:, :, ts(j, P)], wt_n[:, :, ts(j, P)],
                        mybir.ActivationFunctionType.Sign, scale=1.0e30,
                    )
                return wt_n

            # first weight block gates the first matmuls: build it first
            wts = {0: build_wt_block(0)}

            # X path: whole transposed x resident (64KB/partition)
            xt = xt_pool.tile(
                [P, ko_tiles, t_shard], BF16, name="xt", tag="xt"
            )
            for m in range(m_tiles):
                slab = xstage_pool.tile(
                    [P, in_f], BF16, name="xslab", tag="xs"
                )
                nc.gpsimd.dma_start(slab, x[ts(m, P), :])
                nc.sync.dma_start(xt[:, :, ts(m, P)], slab, transpose=True)

            wts[1] = build_wt_block(1)

            bias_rep = bias_pool.tile(
                [P, n_tiles, N_TILE], F32, name="bias_rep", tag="bias"
            )
            for n in range(n_tiles):
                nc.gpsimd.dma_start(
                    bias_rep[:, n, :],
                    b[None, ts(n, N_TILE)].broadcast_to([P, N_TILE]),
                )

            # ---- main loop: n outer (weight stream), m middle, kk inner
            for n in range(n_tiles):
                if n + 2 < n_tiles:
                    wts[n + 2] = build_wt_block(n + 2)
                wt_n = wts.pop(n)
                for m in range(m_tiles):
                    ps = psum_pool.tile([P, N_TILE], F32, name="ps", tag="ps")
                    for kk in range(ko_tiles):
                        nc.tensor.matmul(
                            ps,
                            xt[:, kk, ts(m, P)],
                            wt_n[:, kk, :],
                            start=(kk == 0),
                            stop=(kk == ko_tiles - 1),
                        )
                    out_sb = out_pool.tile(
                        [P, N_TILE], F32, name="out_sb", tag="out_sb"
                    )
                    nc.vector.tensor_tensor(
                        out_sb, ps, bias_rep[:, n, :], mybir.AluOpType.add
                    )
                    nc.scalar.dma_start(
                        out[ts(m, P), ts(n, N_TILE)], out_sb
                    )

    nc.compile()
    return nc


_NC_CACHE = {}


def _get_nc(shape_key):
    if shape_key not in _NC_CACHE:
        _NC_CACHE[shape_key] = build_nc(*shape_key)
    return _NC_CACHE[shape_key]


def kernel(x, weight, bias, _trace=False):
    from concourse.bass_utils import run_bass_kernel_spmd

    x = np.ascontiguousarray(np.asarray(x, dtype=np.float32))
    weight = np.ascontiguousarray(np.asarray(weight, dtype=np.float32))
    bias = np.ascontiguousarray(np.asarray(bias, dtype=np.float32))

    tokens = x.shape[0]
    out_f = weight.shape[0]
    t_shard = tokens // T_GROUPS
    o_shard = out_f // O_GROUPS
    nc = _get_nc((t_shard, x.shape[1], o_shard))

    in_maps = []
    for c in range(N_CORES):
        tg, og = c % T_GROUPS, c // T_GROUPS
        in_maps.append(
            {
                "x": x[tg * t_shard : (tg + 1) * t_shard],
                "weight": weight[og * o_shard : (og + 1) * o_shard],
                "bias": bias[og * o_shard : (og + 1) * o_shard],
            }
        )
    res = run_bass_kernel_spmd(
        nc, in_maps, core_ids=list(range(N_CORES)), trace=_trace
    )
    full = np.empty((tokens, out_f), np.float32)
    for c in range(N_CORES):
        tg, og = c % T_GROUPS, c // T_GROUPS
        full[
            tg * t_shard : (tg + 1) * t_shard,
            og * o_shard : (og + 1) * o_shard,
        ] = res.results[c]["out"]
    if _trace:
        return full, res
    return full


# revision 11
# speedup vs baseline: 1.5672x; 1.1699x over previous
"""BinaryLinear Trainium2 kernel.

Computes out = x @ sign(weight).T + bias for x [8192, 4096] f32,
weight [4096, 4096] f32, bias [4096] f32.

Sharding: data-parallel over tokens across 8 NeuronCores (1024 tokens
per core, weight/bias replicated, no collectives).

Per-core pipeline (chosen so the weight stream fully overlaps compute):
  1. X path (prologue): 8 token slabs are cast f32->bf16 during the
     SWDGE load and XBAR-transposed into a fully SBUF-resident
     XT [128k, 32kk, 1024t] (bf16, 64KB/partition).
  2. W path (streamed): per 512-outfeature block, 4 slabs of 128
     weight rows are cast-loaded, XBAR-transposed into a double-
     buffered WT_n [128k, 32kk, 512o], and signed in place on ScalarE
     (scale=1e30 pushes tiny values off the LUT's zero neighborhood;
     sign(0)=0 preserved). Matmuls for block n start as soon as WT_n
     is signed, while block n+1 loads — the weight stream never gates
     more than one block of compute.
  3. TensorE: for each (n, m): one PSUM bank accumulates 32
     back-to-back matmuls over kk (kk-innermost, one accumulation
     chain per bank, banks rotate only at chain granularity — the
     fastest loop shape measured on HW for N=512 bf16).
  4. DVE adds the partition-broadcast bias while copying PSUM->SBUF;
     the sync-ring HWDGE stores f32 output tiles (gpsimd SWDGE stays
     load-only; ScalarE's queue stays free for the streamed signs).

NOTE: transposes must stay on nc.sync (HWDGE) — issuing them on
nc.scalar's ring corrupts results on HW. Mixed-dtype matmul operands
(bf16 x fp8) hard-wedge the PE (NRT_EXEC_UNIT_UNRECOVERABLE) — both
operands must share a dtype.
"""

import numpy as np

import concourse.mybir as mybir
import concourse.tile as tile
from concourse import bacc
from concourse.bass import ts

P = 128
TOKENS, IN_F, OUT_F = 8192, 4096, 4096
N_CORES = 8
T_GROUPS = 8   # token groups
O_GROUPS = 1   # out-feature groups (weight replicated)
N_TILE = 512   # output-feature block (one PSUM bank of f32)

F32 = mybir.dt.float32
BF16 = mybir.dt.bfloat16


def build_nc(t_shard=TOKENS // T_GROUPS, in_f=IN_F, out_f=OUT_F // O_GROUPS,
             repeat=1):
    m_tiles = t_shard // P       # token tiles of 128
    n_tiles = out_f // N_TILE    # output blocks of 512
    ko_tiles = in_f // P         # contraction tiles of 128
    j_tiles = N_TILE // P        # 128-row weight slabs per block

    nc = bacc.Bacc(None, target_bir_lowering=False, debug=False)

    x = nc.dram_tensor("x", [t_shard, in_f], F32, kind="ExternalInput")
    w = nc.dram_tensor("weight", [out_f, in_f], F32, kind="ExternalInput")
    b = nc.dram_tensor("bias", [out_f], F32, kind="ExternalInput")
    out = nc.dram_tensor("out", [t_shard, out_f], F32, kind="ExternalOutput")

    with tile.TileContext(nc) as tc:
        with (
            tc.tile_pool(name="xt", bufs=1) as xt_pool,
            tc.tile_pool(name="wstage", bufs=4) as wstage_pool,
            tc.tile_pool(name="xstage", bufs=2) as xstage_pool,
            tc.tile_pool(name="wtn", bufs=2) as wtn_pool,
            tc.tile_pool(name="bias", bufs=1) as bias_pool,
            tc.tile_pool(name="out_sb", bufs=3) as out_pool,
            tc.tile_pool(name="ps", bufs=8, space="PSUM") as psum_pool,
        ):
          for _rep in range(repeat):

            def build_wt_block(n):
                wt_n = wtn_pool.tile(
                    [P, ko_tiles, N_TILE], BF16, name=f"wt{n}", tag="wtn"
                )
                for j in range(j_tiles):
                    slab = wstage_pool.tile(
                        [P, in_f], BF16, name="wslab", tag="ws"
                    )
                    nc.gpsimd.dma_start(slab, w[ts(n * j_tiles + j, P), :])
                    nc.sync.dma_start(
                        wt_n[:, :, ts(j, P)], slab, transpose=True
                    )
                    nc.scalar.activation(
                        wt_n[:, :, ts(j, P)], wt_n[:, :, ts(j, P)],
                        mybir.ActivationFunctionType.Sign, scale=1.0e30,
                    )
                return wt_n

            # first weight block gates the first matmuls: build it first
            wts = {0: build_wt_block(0)}

            # X path: whole transposed x resident (64KB/partition)
            xt = xt_pool.tile(
                [P, ko_tiles, t_shard], BF16, name="xt", tag="xt"
            )
            for m in range(m_tiles):
                slab = xstage_pool.tile(
                    [P, in_f], BF16, name="xslab", tag="xs"
                )
                nc.gpsimd.dma_start(slab, x[ts(m, P), :])
                nc.sync.dma_start(xt[:, :, ts(m, P)], slab, transpose=True)

            wts[1] = build_wt_block(1)

            bias_rep = bias_pool.tile(
                [P, n_tiles, N_TILE], F32, name="bias_rep", tag="bias"
            )
            for n in range(n_tiles):
                nc.gpsimd.dma_start(
                    bias_rep[:, n, :],
                    b[None, ts(n, N_TILE)].broadcast_to([P, N_TILE]),
                )

            # ---- main loop: n outer (weight stream), m middle, kk inner
            for n in range(n_tiles):
                if n + 2 < n_tiles:
                    wts[n + 2] = build_wt_block(n + 2)
                wt_n = wts.pop(n)
                for m in range(m_tiles):
                    ps = psum_pool.tile([P, N_TILE], F32, name="ps", tag="ps")
                    for kk in range(ko_tiles):
                        nc.tensor.matmul(
                            ps,
                            xt[:, kk, ts(m, P)],
                            wt_n[:, kk, :],
                            start=(kk == 0),
                            stop=(kk == ko_tiles - 1),
                        )
                    out_sb = out_pool.tile(
                        [P, N_TILE], F32, name="out_sb", tag="out_sb"
                    )
                    nc.vector.tensor_tensor(
                        out_sb, ps, bias_rep[:, n, :], mybir.AluOpType.add
                    )
                    # stores on the SP ring: ScalarE's NX stays free for
                    # the streamed sign activations
                    nc.sync.dma_start(
                        out[ts(m, P), ts(n, N_TILE)], out_sb
                    )

    nc.compile()
    return nc


_NC_CACHE = {}


def _get_nc(shape_key):
    if shape_key not in _NC_CACHE:
        _NC_CACHE[shape_key] = build_nc(*shape_key)
    return _NC_CACHE[shape_key]


def kernel(x, weight, bias, _trace=False):
    from concourse.bass_utils import run_bass_kernel_spmd

    x = np.ascontiguousarray(np.asarray(x, dtype=np.float32))
    weight = np.ascontiguousarray(np.asarray(weight, dtype=np.float32))
    bias = np.ascontiguousarray(np.asarray(bias, dtype=np.float32))

    tokens = x.shape[0]
    out_f = weight.shape[0]
    t_shard = tokens // T_GROUPS
    o_shard = out_f // O_GROUPS
    nc = _get_nc((t_shard, x.shape[1], o_shard))

    in_maps = []
    for c in range(N_CORES):
        tg, og = c % T_GROUPS, c // T_GROUPS
        in_maps.append(
            {
                "x": x[tg * t_shard : (tg + 1) * t_shard],
                "weight": weight[og * o_shard : (og + 1) * o_shard],
                "bias": bias[og * o_shard : (og + 1) * o_shard],
            }
        )
    res = run_bass_kernel_spmd(
        nc, in_maps, core_ids=list(range(N_CORES)), trace=_trace
    )
    full = np.empty((tokens, out_f), np.float32)
    for c in range(N_CORES):
        tg, og = c % T_GROUPS, c // T_GROUPS
        full[
            tg * t_shard : (tg + 1) * t_shard,
            og * o_shard : (og + 1) * o_shard,
        ] = res.results[c]["out"]
    if _trace:
        return full, res
    return full
